# revision 9
# baseline (speedup 1.0000x reference)
"""KronEmbedding lookup kernel for 8 TRN2 NeuronCores.

Math: w = einsum('sia,sjb->ijab', A, B).reshape(50176, 2048); out = w[x].
Never materializes w. Per token t with i=x//224, j=x%224:
    out[t] = sum_s outer(A[s,i,:], B[s,j,:])   -> (64*32 = 2048 floats)

Strategy (data-parallel over tokens, 1024 tokens/core, all bf16 on the wire):
- Tokens in 64 groups of 16 (k in [0,16)); contraction partition p = 8k+s.
- Per group, two overlapping sub-array matmuls (tile_position (0,0)/(64,64)):
  contraction rows 64*hh..64*hh+64 x stationary AG rows -> out partitions
  (hh, a); moving operand bd[64*hh.., g, :] is the block-diagonal
  ([256] = 8 tokens x 32). Both operands CONTIGUOUS per group (strided rhs
  is 4x slower on HW).
- bd is 8x zero-padded (4 MiB); shipping it all costs 11.6us of DMA bus.
  Hybrid build:
    groups 0..31  pre-padded from HBM (2 MiB) so matmuls start immediately;
    groups 32..63 zeroed on-device via 8 per-kk-column strip memsets
                  (DVE/ACT/Pool) and compact B rows (0.25 MiB) scattered
                  into the diagonal blocks by 16 DMAs. A scatter only
                  depends on its own kk strip memset, so scatters flow
                  while the pre-padded groups compute.
- Queue discipline (in-order queues; every sem wait blocks the queue):
  sync carries input loads + 10 scatters; gpsimd (SWDGE) 6 scatters;
  scalar carries only evac-paced out-DMAs + AG tail; evacuations are
  split per-ps-tile into DVE half + ACT half so both engines pace the
  PSUM drain without either queue waiting on scatters.
- Host: upcast bf16 -> fp32 and reorder to token-major (untimed).
"""
import numpy as np
import ml_dtypes
from contextlib import ExitStack

import concourse.bass as bass
import concourse.bacc as bacc
import concourse.tile as tile
import concourse.mybir as mybir
from concourse import bass_utils

dt = mybir.dt
BF16 = ml_dtypes.bfloat16

R, M1, N1, M2, N2 = 8, 224, 64, 224, 32
VOCAB, EMB = M1 * M2, N1 * N2          # 50176, 2048
BATCH, SEQ = 4, 2048
NTOK = BATCH * SEQ                     # 8192
NCORES = 8
TPC = NTOK // NCORES                   # 1024 tokens per core
NGRP = TPC // 16                       # 64 groups of 16 tokens
QP = 32                                # pre-padded leading groups
NREST = NGRP - QP                      # scatter-built groups
NWARM = 4

_CACHE = {}


def _build():
    nc = bacc.Bacc("TRN2", num_devices=NCORES)
    AG = nc.dram_tensor("AG", [128, NGRP, 64], dt.bfloat16, kind="ExternalInput")
    BDF = nc.dram_tensor("BDF", [128, QP, 256], dt.bfloat16, kind="ExternalInput")
    GBR = nc.dram_tensor("GBR", [8, 2, 8, NREST, 32], dt.bfloat16,
                         kind="ExternalInput")
    out = nc.dram_tensor("out", [8, 128, 2048], dt.bfloat16, kind="ExternalOutput")

    with tile.TileContext(nc) as tc, ExitStack() as ctx:
        const_pool = ctx.enter_context(tc.tile_pool(name="const", bufs=1))
        ev_pool = ctx.enter_context(tc.tile_pool(name="ev", bufs=3))
        ps_pool = ctx.enter_context(tc.tile_pool(name="ps", bufs=3, space="PSUM"))
        wps_pool = ctx.enter_context(tc.tile_pool(name="wps", bufs=1, space="PSUM"))

        warm = const_pool.tile([128, 512], dt.bfloat16, tag="warm")
        nc.vector.memset(warm[:], 0.0)
        wps = wps_pool.tile([128, 512], dt.float32, tag="wps")
        for _ in range(NWARM):
            nc.tensor.matmul(wps[:], warm[:, 0:128], warm[:], start=True, stop=True)

        ag = const_pool.tile([128, NGRP, 64], dt.bfloat16, tag="ag", name="ag")
        bd = const_pool.tile([128, NGRP, 256], dt.bfloat16, tag="bd", name="bd")

        # Input loads, earliest (transfers drain during the runtime preamble).
        nc.sync.dma_start(ag[:, 0:32], AG[:, 0:32])
        nc.sync.dma_start(bd[:, 0:16], BDF[:, 0:16])
        nc.sync.dma_start(bd[:, 16:QP], BDF[:, 16:QP])
        nc.scalar.dma_start(ag[:, 32:NGRP], AG[:, 32:NGRP])

        # Per-kk-column strip memsets of the scatter-built region: a scatter
        # (kk, xh) only overlaps strip kk.
        strip_engine = [nc.vector, nc.vector, nc.vector, nc.gpsimd,
                        nc.gpsimd, nc.gpsimd, nc.scalar, nc.scalar]
        for kk in range(8):
            eng = strip_engine[kk]
            ap = bd[:, QP:NGRP, 32 * kk:32 * kk + 32]
            if eng is nc.scalar:
                eng.memzero(ap)
            else:
                eng.memset(ap, 0.0)

        # Scatters: sync kk 0-4, gpsimd kk 5-7 (scalar stays unblocked).
        for kk in range(8):
            for xh in range(2):
                (nc.sync if kk < 5 else nc.gpsimd).dma_start(
                    bd[64 * xh + 8 * kk:64 * xh + 8 * kk + 8,
                       QP:NGRP, 32 * kk:32 * kk + 32],
                    GBR[kk, xh],
                )

        # Main stream: 8 chunks x 8 groups; 2 sub-array matmuls per group.
        for chunk in range(8):
            ev = ev_pool.tile([128, 2048], dt.bfloat16, tag="ev")
            for h2 in range(2):
                ps = ps_pool.tile([128, 1024], dt.float32, tag="ps")
                for h in range(4):
                    g = 8 * chunk + 4 * h2 + h
                    for hh in range(2):
                        nc.tensor.matmul(
                            ps[64 * hh:64 * hh + 64, 256 * h:256 * h + 256],
                            ag[64 * hh:64 * hh + 64, g, :],
                            bd[64 * hh:64 * hh + 64, g, :],
                            start=True,
                            stop=True,
                            tile_position=(64 * hh, 64 * hh),
                        )
                # Split evacuation: DVE takes one half, ACT the other.
                base = 1024 * h2
                nc.vector.tensor_copy(ev[:, base:base + 512], ps[:, 0:512])
                nc.scalar.copy(ev[:, base + 512:base + 1024], ps[:, 512:1024])
            nc.scalar.dma_start(out[chunk], ev[:])

    nc.compile()
    return nc


def kernel(A: np.ndarray, B: np.ndarray, x: np.ndarray) -> np.ndarray:
    Abf = np.asarray(A, dtype=np.float32).astype(BF16)    # [8, 224, 64]
    Bbf = np.asarray(B, dtype=np.float32).astype(BF16)    # [8, 224, 32]
    xl = np.asarray(x).astype(np.int64).reshape(-1)       # [8192]
    i_all = (xl // M2).astype(np.int64)
    j_all = (xl % M2).astype(np.int64)

    if "nc" not in _CACHE:
        _CACHE["nc"] = _build()
    nc = _CACHE["nc"]

    in_maps = []
    for c in range(NCORES):
        sl = slice(c * TPC, (c + 1) * TPC)
        IA = i_all[sl].reshape(NGRP, 16)                  # [g, k]
        JB = j_all[sl].reshape(NGRP, 16)

        # AG[p, g, a] = A[s, i_t, a], p = 8k+s, t = 16g+k (compact lhsT)
        AGh = np.ascontiguousarray(
            Abf[:, IA, :].transpose(2, 0, 1, 3)           # [16k, 8s, g, a]
        ).reshape(128, NGRP, 64)

        # GB[k, s, g, b] = B[s, j_t, b]
        GB = Bbf[:, JB, :].transpose(2, 0, 1, 3)          # [16k, 8s, g, b]

        # BDF: pre-padded block-diagonal for the first QP groups.
        BDFh = np.zeros((16, 8, QP, 8, 32), dtype=BF16)   # [k, s, g, k8, b]
        for k in range(16):
            BDFh[k, :, :, k % 8, :] = GB[k, :, 0:QP, :]
        BDFh = BDFh.reshape(128, QP, 256)

        # GBR[kk, xh, s, grest, b] = B row of token k = 8*xh + kk.
        GBRh = np.ascontiguousarray(
            GB[:, :, QP:NGRP, :].reshape(2, 8, 8, NREST, 32).transpose(1, 0, 2, 3, 4)
        )
        in_maps.append(dict(AG=AGh, BDF=BDFh, GBR=GBRh))

    _CACHE["in_maps"] = in_maps
    res = bass_utils.run_bass_kernel_spmd(nc, in_maps, core_ids=list(range(NCORES)))

    outs = []
    for c in range(NCORES):
        o = np.asarray(res.results[c]["out"]).astype(np.float32)  # [8,128,2048]
        # rows: (hh, a); cols within chunk: (h2, h, k8, b), g = 8*chunk+4*h2+h
        o = o.reshape(8, 2, 64, 2, 4, 8, 32)             # [chunk, hh, a, h2, h, k8, b]
        # token t = 16*g + 8*hh + k8 = 128*chunk + 16*(4*h2+h) + 8*hh + k8
        o = o.transpose(0, 3, 4, 1, 5, 2, 6)             # [chunk, h2, h, hh, k8, a, b]
        outs.append(o.reshape(TPC, EMB))
    full = np.concatenate(outs, axis=0)                  # [8192, 2048]
    return full.reshape(BATCH, SEQ, EMB)
